# revision 15
# baseline (speedup 1.0000x reference)
"""Trainium2 Bass kernel for nn_DiffForest (soft decision forest forward).

Math: per tree t, z = x @ w_d[t]; p = sigmoid(z); leaf path probs are products
of 8 factors p/(1-p) down a depth-8 tree; output = sum_t leaf_prob @ softmax(w_l[t]) / 10.

Kernel formulation:
  - The 512 "leaves" come in identical pairs -> fold to 256 paths; fold the
    pair-sum + 1/n_trees into the leaf weight matrix w2 (host, exact).
  - Decision matmul in fp8e4 DoubleRow mode (K=256 per instruction, 2x bf16
    throughput). w_d scaled x64 on host; the 1/64 unscale is folded into the
    activation scale and the S-matrix z rows.
  - Path products in log space over the first 7 levels only: the 127 internal
    nodes feed a [256 -> 128] matmul (constant 0/1 S7 matrix, fp32r) giving
    B[k] = -log P(parent path k). Decision nodes are permuted on host so the
    128 level-8 nodes land on partitions aligned with their parent path:
        lp[2k]   = exp(-(B[k] + softplus(-z8[k])))          (ACT)
        lp[2k+1] = lp[2k] * exp(-z8[k])                     (DVE multiply)
    This shrinks the S-matmul 4x vs the 8-level [512 -> 256] version and
    moves the last level to otherwise-idle engines.
  - softplus(-z) = ln(1 + exp(-z)) via Exp/Ln (one ACT table set, no swaps).
  - Leaf matmul in bf16 (fp8 would sit right at the error gate).
  - Sharding: data-parallel over batch; each of the 8 cores takes 2048 rows,
    weights replicated, no collectives.
"""

import numpy as np
import ml_dtypes

import concourse.bacc as bacc
import concourse.mybir as mybir
import concourse.tile as tile
from concourse.bass_utils import run_bass_kernel_spmd

N_CORES = 8
BATCH = 16384
B_LOC = BATCH // N_CORES        # 2048 rows per core
IN_DIM = 2048
N_TREES = 10
ND_PAD = 256                    # decision nodes padded 255 -> 256 (permuted)
CLASSES = 1000
CHUNK = 512                     # batch columns processed per chunk
N_CHUNKS = B_LOC // CHUNK
KJ = IN_DIM // 256              # 8 DoubleRow contraction tiles

BF16 = mybir.dt.bfloat16
F32 = mybir.dt.float32
F32R = mybir.dt.float32r
F16 = mybir.dt.float16
F8E4 = mybir.dt.float8e4
AF = mybir.ActivationFunctionType
DR = mybir.MatmulPerfMode.DoubleRow

W_SCALE = 64.0

_CACHE = {}


def _build(n_trees=N_TREES):
    nc = bacc.Bacc("TRN2", target_bir_lowering=False)
    xt = nc.dram_tensor("xt", (N_CHUNKS, 128, KJ, 2, CHUNK), F8E4, kind="ExternalInput")
    wd = nc.dram_tensor("wd", (n_trees, 128, KJ, 2, ND_PAD), F8E4, kind="ExternalInput")
    smat = nc.dram_tensor("smat", (2, 128, 128), F32R, kind="ExternalInput")
    w2 = nc.dram_tensor("w2", (n_trees, 128, 2, CLASSES), BF16, kind="ExternalInput")
    out = nc.dram_tensor("out", (B_LOC, CLASSES), F32, kind="ExternalOutput")

    with tile.TileContext(nc) as tc:
        with (
            tc.tile_pool(name="const", bufs=1) as constp,
            tc.tile_pool(name="xp", bufs=2) as xp,
            tc.tile_pool(name="ep", bufs=2) as ep,
            tc.tile_pool(name="spp", bufs=2) as spp,
            tc.tile_pool(name="zp", bufs=2) as zp,
            tc.tile_pool(name="a0p", bufs=2) as a0p,
            tc.tile_pool(name="outp", bufs=2) as outp,
            tc.tile_pool(name="lptp", bufs=2) as lptp,
            tc.tile_pool(name="pz", bufs=3, space="PSUM") as pzp,
            tc.tile_pool(name="pb", bufs=1, space="PSUM") as pbp,
            tc.tile_pool(name="poa", bufs=2, space="PSUM") as poap,
            tc.tile_pool(name="pob", bufs=2, space="PSUM") as pobp,
        ):
            wd_sb = constp.tile([128, n_trees, KJ, 2, ND_PAD], F8E4)
            w2_sb = constp.tile([128, n_trees, 2, CLASSES], BF16)
            smat_sb = constp.tile([128, 2, 128], F32R)

            # Pin the combined exp+ln table set (id 6,
            # natural_log_exp_and_others): without this the table-load pass
            # alternates exp_and_others / natural_log on every Exp<->Ln
            # transition (~1.5us per load, ~80 loads).
            nc.scalar.add_instruction(
                mybir.InstLoadActFuncSet(
                    name=nc.get_next_instruction_name(),
                    act_func_set_id=6, ins=[], outs=[],
                )
            )

            # startup order: first halves of x chunk 0 + tree-0 weights go
            # first so the PE starts as early as possible
            started = False

            def emit_mm2(lpT):
                for s in range(CHUNK // 128):
                    po_a = poap.tile([128, 512], F32, tag="poa")
                    po_b = pobp.tile([128, 512], F32, tag="pob")
                    n_acc = n_trees * 2
                    i = 0
                    for t in range(n_trees):
                        for lt in range(2):
                            first = i == 0
                            last = i == n_acc - 1
                            lhsT = lpT[:, t, lt, s * 128 : (s + 1) * 128]
                            nc.tensor.matmul(
                                po_a[:, 0:500], lhsT, w2_sb[:, t, lt, 0:500],
                                start=first, stop=last,
                            )
                            nc.tensor.matmul(
                                po_b[:, 0:500], lhsT, w2_sb[:, t, lt, 500:1000],
                                start=first, stop=last,
                            )
                            i += 1
                    osb = outp.tile([128, CLASSES], F32, tag="osb")
                    nc.vector.tensor_copy(osb[:, 0:500], po_a[:, 0:500])
                    yield s, 0, osb
                    nc.vector.tensor_copy(osb[:, 500:1000], po_b[:, 0:500])
                    yield s, 1, osb

            for ci in range(N_CHUNKS):
                xsb = xp.tile([128, KJ, 2, CHUNK], F8E4, tag="xsb")
                if ci == 0:
                    # x pieces on the sync DGE queue, weights on the (idle)
                    # gpsimd queue: the two transfer chains start in parallel
                    nc.sync.dma_start(xsb[:, 0:2, :, :], xt[ci, :, 0:2, :, :])
                    nc.gpsimd.dma_start(wd_sb[:, 0, 0:2, :, :], wd[0, :, 0:2, :, :])
                    nc.sync.dma_start(xsb[:, 2:8, :, :], xt[ci, :, 2:8, :, :])
                    nc.gpsimd.dma_start(wd_sb[:, 0, 2:8, :, :], wd[0, :, 2:8, :, :])
                    nc.gpsimd.dma_start(
                        smat_sb[:, :, :], smat[:, :, :].rearrange("k p m -> p k m")
                    )
                else:
                    nc.sync.dma_start(xsb[:, :, :, :], xt[ci, :, :, :, :])
                lpT = lptp.tile([128, n_trees, 2, CHUNK], BF16, tag="lpT")
                for t in range(n_trees):
                    # decision matmuls: fp8 DoubleRow, K=256 per instruction
                    psz_i = pzp.tile([128, CHUNK], F32, tag="psz")
                    psz_l = pzp.tile([128, CHUNK], F32, tag="psz")
                    # nh outer: PSUM accumulation groups in the same bank must
                    # not interleave (corrupts accumulation)
                    for dt_, pt in ((0, psz_i), (1, psz_l)):
                        for nh in range(2):
                            for j in range(KJ):
                                nc.tensor.matmul(
                                    pt[:, nh * 256 : (nh + 1) * 256],
                                    wd_sb[:, t, j, :, dt_ * 128 : (dt_ + 1) * 128],
                                    xsb[:, j, :, nh * 256 : (nh + 1) * 256],
                                    start=(j == 0), stop=(j == KJ - 1),
                                    perf_mode=DR,
                                )
                    if not started:
                        # queue the bulk weight DMAs behind the first tree's
                        # matmul emissions
                        for tt in range(1, n_trees):
                            nc.sync.dma_start(wd_sb[:, tt, :, :, :], wd[tt, :, :, :, :])
                        for tt in range(n_trees):
                            nc.sync.dma_start(w2_sb[:, tt, :, :], w2[tt, :, :, :])
                        started = True
                    # E = [exp(-z_int); exp(-z8)]; psum holds 64*z
                    E = ep.tile([128, 2, CHUNK], F16, tag="E")
                    nc.scalar.activation(E[:, 0, :], psz_i[:, :], AF.Exp, scale=-1.0 / W_SCALE)
                    nc.scalar.activation(E[:, 1, :], psz_l[:, :], AF.Exp, scale=-1.0 / W_SCALE)
                    # SP = [softplus(-z_int); softplus(-z8)] = ln(E + 1)
                    SP = spp.tile([128, 2, CHUNK], F32R, tag="SP")
                    nc.scalar.activation(SP[:, :, :], E[:, :, :], AF.Ln, bias=1.0)
                    ZI = zp.tile([128, CHUNK], F32R, tag="ZI")
                    nc.vector.tensor_copy(ZI[:, :], psz_i[:, :])
                    # B[k] = -log P(parent path k) over levels 1-7
                    Bp = pbp.tile([128, CHUNK], F32, tag="B")
                    nc.tensor.matmul(Bp[:, :], smat_sb[:, 0, :], SP[:, 0, :], start=True, stop=False)
                    nc.tensor.matmul(Bp[:, :], smat_sb[:, 1, :], ZI[:, :], start=False, stop=True)
                    A0 = a0p.tile([128, CHUNK], F32, tag="A0")
                    nc.vector.tensor_add(A0[:, :], Bp[:, :], SP[:, 1, :])
                    nc.scalar.activation(lpT[:, t, 0, :], A0[:, :], AF.Exp, scale=-1.0)
                    nc.vector.tensor_mul(lpT[:, t, 1, :], lpT[:, t, 0, :], E[:, 1, :])
                c0 = ci * CHUNK
                for s, half, osb in emit_mm2(lpT):
                    nc.sync.dma_start(
                        out[c0 + s * 128 : c0 + (s + 1) * 128, half * 500 : (half + 1) * 500],
                        osb[:, half * 500 : (half + 1) * 500],
                    )
    nc.compile()
    return nc


def _smat_np():
    # S7 [2, 128, 128]: k-tile 0 = softplus rows, k-tile 1 = z rows (x 1/64)
    S = np.zeros((2, 128, 128), np.float32)
    for k in range(128):
        for n in range(7):
            node = (2**n - 1) + (k >> (7 - n))
            branch = (k >> (6 - n)) & 1
            S[0, node, k] += 1.0
            S[1, node, k] += branch / W_SCALE
    return S


def _prep_weights(w_d, w_l, n_trees=N_TREES):
    bf16 = ml_dtypes.bfloat16
    e4m3 = ml_dtypes.float8_e4m3
    w_l = np.asarray(w_l, dtype=np.float32)
    m = w_l.max(axis=-1, keepdims=True)
    e = np.exp(w_l - m, dtype=np.float32)
    sm = e / e.sum(axis=-1, keepdims=True)
    w2 = (sm[:, 0::2, :] + sm[:, 1::2, :]) * np.float32(1.0 / n_trees)  # [T, 256, C]
    # split by last-level branch: w2h[t, k, 0] = path 2k, w2h[t, k, 1] = path 2k+1
    w2h = np.stack([w2[:, 0::2, :], w2[:, 1::2, :]], axis=2).astype(bf16)

    # node permutation: slots 0..126 = internal nodes 0..126, slot 127 = pad,
    # slots 128..255 = level-8 nodes 127..254 (slot 128+k = node 127+k)
    wd_p = np.zeros((n_trees, IN_DIM, ND_PAD), np.float32)
    wd_p[:, :, 0:127] = w_d[:, :, 0:127]
    wd_p[:, :, 128:256] = w_d[:, :, 127:255]
    wd_p *= np.float32(W_SCALE)
    # DoubleRow layout [t, p, j, i, m]: in_dim = 256j + 128i + p
    wd8 = np.ascontiguousarray(
        wd_p.reshape(n_trees, KJ, 2, 128, ND_PAD).transpose(0, 3, 1, 2, 4)
    ).astype(e4m3)
    return wd8, _smat_np(), w2h


last_bass_results = None


def kernel(x, w_d, w_l):
    global last_bass_results
    x = np.asarray(x, dtype=np.float32)
    wd8, S, w2h = _prep_weights(np.asarray(w_d, dtype=np.float32), w_l)
    e4m3 = ml_dtypes.float8_e4m3
    in_maps = []
    for c in range(N_CORES):
        xc = x[c * B_LOC : (c + 1) * B_LOC, :].T  # [in_dim, b_loc]
        xh = np.ascontiguousarray(
            xc.reshape(KJ, 2, 128, N_CHUNKS, CHUNK).transpose(3, 2, 0, 1, 4)
        ).astype(e4m3)
        in_maps.append({"xt": xh, "wd": wd8, "smat": S, "w2": w2h})
    if "nc" not in _CACHE:
        _CACHE["nc"] = _build()
    res = run_bass_kernel_spmd(_CACHE["nc"], in_maps, core_ids=list(range(N_CORES)))
    last_bass_results = res
    return np.concatenate([res.results[c]["out"] for c in range(N_CORES)], axis=0)


# revision 16
# speedup vs baseline: 1.0176x; 1.0176x over previous
"""Trainium2 Bass kernel for nn_DiffForest (soft decision forest forward).

Math: per tree t, z = x @ w_d[t]; p = sigmoid(z); leaf path probs are products
of 8 factors p/(1-p) down a depth-8 tree; output = sum_t leaf_prob @ softmax(w_l[t]) / 10.

Kernel formulation:
  - The 512 "leaves" come in identical pairs -> fold to 256 paths; fold the
    pair-sum + 1/n_trees into the leaf weight matrix w2 (host, exact).
  - Decision matmul in fp8e4 DoubleRow mode (K=256 per instruction, 2x bf16
    throughput). w_d scaled x64 on host; the 1/64 unscale is folded into the
    activation scale and the S-matrix z rows.
  - Path products in log space over the first 7 levels only: the 127 internal
    nodes feed a [256 -> 128] matmul (constant 0/1 S7 matrix, fp32r) giving
    B[k] = -log P(parent path k). Decision nodes are permuted on host so the
    128 level-8 nodes land on partitions aligned with their parent path:
        lp[2k]   = exp(-(B[k] + softplus(-z8[k])))          (ACT)
        lp[2k+1] = lp[2k] * exp(-z8[k])                     (DVE multiply)
    This shrinks the S-matmul 4x vs the 8-level [512 -> 256] version and
    moves the last level to otherwise-idle engines.
  - softplus(-z) = ln(1 + exp(-z)) via Exp/Ln (one ACT table set, no swaps).
  - Leaf matmul in bf16 (fp8 would sit right at the error gate).
  - Sharding: data-parallel over batch; each of the 8 cores takes 2048 rows,
    weights replicated, no collectives.
"""

import numpy as np
import ml_dtypes

import concourse.bacc as bacc
import concourse.mybir as mybir
import concourse.tile as tile
from concourse.bass_utils import run_bass_kernel_spmd

N_CORES = 8
BATCH = 16384
B_LOC = BATCH // N_CORES        # 2048 rows per core
IN_DIM = 2048
N_TREES = 10
ND_PAD = 256                    # decision nodes padded 255 -> 256 (permuted)
CLASSES = 1000
CHUNK = 512                     # batch columns processed per chunk
N_CHUNKS = B_LOC // CHUNK
KJ = IN_DIM // 256              # 8 DoubleRow contraction tiles

BF16 = mybir.dt.bfloat16
F32 = mybir.dt.float32
F32R = mybir.dt.float32r
F16 = mybir.dt.float16
F8E4 = mybir.dt.float8e4
AF = mybir.ActivationFunctionType
DR = mybir.MatmulPerfMode.DoubleRow

W_SCALE = 64.0

_CACHE = {}


def _build(n_trees=N_TREES):
    nc = bacc.Bacc("TRN2", target_bir_lowering=False)
    xt = nc.dram_tensor("xt", (N_CHUNKS, 128, KJ, 2, CHUNK), F8E4, kind="ExternalInput")
    wd = nc.dram_tensor("wd", (n_trees, 128, KJ, 2, ND_PAD), F8E4, kind="ExternalInput")
    smat = nc.dram_tensor("smat", (2, 128, 128), F32R, kind="ExternalInput")
    w2 = nc.dram_tensor("w2", (n_trees, 128, 2, CLASSES), BF16, kind="ExternalInput")
    out = nc.dram_tensor("out", (B_LOC, CLASSES), F32, kind="ExternalOutput")

    with tile.TileContext(nc) as tc:
        with (
            tc.tile_pool(name="const", bufs=1) as constp,
            tc.tile_pool(name="xp", bufs=2) as xp,
            tc.tile_pool(name="ep", bufs=2) as ep,
            tc.tile_pool(name="spp", bufs=2) as spp,
            tc.tile_pool(name="zp", bufs=2) as zp,
            tc.tile_pool(name="a0p", bufs=2) as a0p,
            tc.tile_pool(name="outp", bufs=2) as outp,
            tc.tile_pool(name="lptp", bufs=2) as lptp,
            tc.tile_pool(name="pz", bufs=3, space="PSUM") as pzp,
            tc.tile_pool(name="pb", bufs=1, space="PSUM") as pbp,
            tc.tile_pool(name="poa", bufs=2, space="PSUM") as poap,
            tc.tile_pool(name="pob", bufs=2, space="PSUM") as pobp,
        ):
            wd_sb = constp.tile([128, n_trees, KJ, 2, ND_PAD], F8E4)
            w2_sb = constp.tile([128, n_trees, 2, CLASSES], BF16)
            smat_sb = constp.tile([128, 2, 128], F32R)

            # Pin the combined exp+ln table set (id 6,
            # natural_log_exp_and_others): without this the table-load pass
            # alternates exp_and_others / natural_log on every Exp<->Ln
            # transition (~1.5us per load, ~80 loads).
            nc.scalar.add_instruction(
                mybir.InstLoadActFuncSet(
                    name=nc.get_next_instruction_name(),
                    act_func_set_id=6, ins=[], outs=[],
                )
            )

            # startup order: first halves of x chunk 0 + tree-0 weights go
            # first so the PE starts as early as possible
            started = False

            def emit_mm2(lpT):
                for s in range(CHUNK // 128):
                    po_a = poap.tile([128, 512], F32, tag="poa")
                    po_b = pobp.tile([128, 512], F32, tag="pob")
                    n_acc = n_trees * 2
                    i = 0
                    for t in range(n_trees):
                        for lt in range(2):
                            first = i == 0
                            last = i == n_acc - 1
                            lhsT = lpT[:, t, lt, s * 128 : (s + 1) * 128]
                            nc.tensor.matmul(
                                po_a[:, 0:500], lhsT, w2_sb[:, t, lt, 0:500],
                                start=first, stop=last,
                            )
                            nc.tensor.matmul(
                                po_b[:, 0:500], lhsT, w2_sb[:, t, lt, 500:1000],
                                start=first, stop=last,
                            )
                            i += 1
                    osb = outp.tile([128, CLASSES], F32, tag="osb")
                    nc.vector.tensor_copy(osb[:, 0:500], po_a[:, 0:500])
                    yield s, 0, osb
                    nc.vector.tensor_copy(osb[:, 500:1000], po_b[:, 0:500])
                    yield s, 1, osb

            for ci in range(N_CHUNKS):
                xsb = xp.tile([128, KJ, 2, CHUNK], F8E4, tag="xsb")
                if ci == 0:
                    nc.sync.dma_start(xsb[:, 0:2, :, :], xt[ci, :, 0:2, :, :])
                    nc.sync.dma_start(wd_sb[:, 0, 0:2, :, :], wd[0, :, 0:2, :, :])
                    nc.sync.dma_start(xsb[:, 2:8, :, :], xt[ci, :, 2:8, :, :])
                    nc.sync.dma_start(wd_sb[:, 0, 2:8, :, :], wd[0, :, 2:8, :, :])
                    nc.sync.dma_start(
                        smat_sb[:, :, :], smat[:, :, :].rearrange("k p m -> p k m")
                    )
                else:
                    nc.sync.dma_start(xsb[:, :, :, :], xt[ci, :, :, :, :])
                lpT = lptp.tile([128, n_trees, 2, CHUNK], BF16, tag="lpT")
                for t in range(n_trees):
                    # decision matmuls: fp8 DoubleRow, K=256 per instruction
                    psz_i = pzp.tile([128, CHUNK], F32, tag="psz")
                    psz_l = pzp.tile([128, CHUNK], F32, tag="psz")
                    # nh outer: PSUM accumulation groups in the same bank must
                    # not interleave (corrupts accumulation)
                    for dt_, pt in ((0, psz_i), (1, psz_l)):
                        for nh in range(2):
                            for j in range(KJ):
                                nc.tensor.matmul(
                                    pt[:, nh * 256 : (nh + 1) * 256],
                                    wd_sb[:, t, j, :, dt_ * 128 : (dt_ + 1) * 128],
                                    xsb[:, j, :, nh * 256 : (nh + 1) * 256],
                                    start=(j == 0), stop=(j == KJ - 1),
                                    perf_mode=DR,
                                )
                    if not started:
                        # queue the bulk weight DMAs behind the first tree's
                        # matmul emissions
                        for tt in range(1, n_trees):
                            nc.sync.dma_start(wd_sb[:, tt, :, :, :], wd[tt, :, :, :, :])
                        for tt in range(n_trees):
                            nc.sync.dma_start(w2_sb[:, tt, :, :], w2[tt, :, :, :])
                        started = True
                    # E = [exp(-z_int); exp(-z8)]; psum holds 64*z
                    E = ep.tile([128, 2, CHUNK], F16, tag="E")
                    nc.scalar.activation(E[:, 0, :], psz_i[:, :], AF.Exp, scale=-1.0 / W_SCALE)
                    nc.scalar.activation(E[:, 1, :], psz_l[:, :], AF.Exp, scale=-1.0 / W_SCALE)
                    # SP = [softplus(-z_int); softplus(-z8)] = ln(E + 1)
                    SP = spp.tile([128, 2, CHUNK], F32R, tag="SP")
                    nc.scalar.activation(SP[:, :, :], E[:, :, :], AF.Ln, bias=1.0)
                    ZI = zp.tile([128, CHUNK], F32R, tag="ZI")
                    nc.vector.tensor_copy(ZI[:, :], psz_i[:, :])
                    # B[k] = -log P(parent path k) over levels 1-7
                    Bp = pbp.tile([128, CHUNK], F32, tag="B")
                    nc.tensor.matmul(Bp[:, :], smat_sb[:, 0, :], SP[:, 0, :], start=True, stop=False)
                    nc.tensor.matmul(Bp[:, :], smat_sb[:, 1, :], ZI[:, :], start=False, stop=True)
                    A0 = a0p.tile([128, CHUNK], F32, tag="A0")
                    nc.vector.tensor_add(A0[:, :], Bp[:, :], SP[:, 1, :])
                    nc.scalar.activation(lpT[:, t, 0, :], A0[:, :], AF.Exp, scale=-1.0)
                    nc.vector.tensor_mul(lpT[:, t, 1, :], lpT[:, t, 0, :], E[:, 1, :])
                c0 = ci * CHUNK
                for s, half, osb in emit_mm2(lpT):
                    nc.sync.dma_start(
                        out[c0 + s * 128 : c0 + (s + 1) * 128, half * 500 : (half + 1) * 500],
                        osb[:, half * 500 : (half + 1) * 500],
                    )
    nc.compile()
    return nc


def _smat_np():
    # S7 [2, 128, 128]: k-tile 0 = softplus rows, k-tile 1 = z rows (x 1/64)
    S = np.zeros((2, 128, 128), np.float32)
    for k in range(128):
        for n in range(7):
            node = (2**n - 1) + (k >> (7 - n))
            branch = (k >> (6 - n)) & 1
            S[0, node, k] += 1.0
            S[1, node, k] += branch / W_SCALE
    return S


def _prep_weights(w_d, w_l, n_trees=N_TREES):
    bf16 = ml_dtypes.bfloat16
    e4m3 = ml_dtypes.float8_e4m3
    w_l = np.asarray(w_l, dtype=np.float32)
    m = w_l.max(axis=-1, keepdims=True)
    e = np.exp(w_l - m, dtype=np.float32)
    sm = e / e.sum(axis=-1, keepdims=True)
    w2 = (sm[:, 0::2, :] + sm[:, 1::2, :]) * np.float32(1.0 / n_trees)  # [T, 256, C]
    # split by last-level branch: w2h[t, k, 0] = path 2k, w2h[t, k, 1] = path 2k+1
    w2h = np.stack([w2[:, 0::2, :], w2[:, 1::2, :]], axis=2).astype(bf16)

    # node permutation: slots 0..126 = internal nodes 0..126, slot 127 = pad,
    # slots 128..255 = level-8 nodes 127..254 (slot 128+k = node 127+k)
    wd_p = np.zeros((n_trees, IN_DIM, ND_PAD), np.float32)
    wd_p[:, :, 0:127] = w_d[:, :, 0:127]
    wd_p[:, :, 128:256] = w_d[:, :, 127:255]
    wd_p *= np.float32(W_SCALE)
    # DoubleRow layout [t, p, j, i, m]: in_dim = 256j + 128i + p
    wd8 = np.ascontiguousarray(
        wd_p.reshape(n_trees, KJ, 2, 128, ND_PAD).transpose(0, 3, 1, 2, 4)
    ).astype(e4m3)
    return wd8, _smat_np(), w2h


last_bass_results = None


def kernel(x, w_d, w_l):
    global last_bass_results
    x = np.asarray(x, dtype=np.float32)
    wd8, S, w2h = _prep_weights(np.asarray(w_d, dtype=np.float32), w_l)
    e4m3 = ml_dtypes.float8_e4m3
    in_maps = []
    for c in range(N_CORES):
        xc = x[c * B_LOC : (c + 1) * B_LOC, :].T  # [in_dim, b_loc]
        xh = np.ascontiguousarray(
            xc.reshape(KJ, 2, 128, N_CHUNKS, CHUNK).transpose(3, 2, 0, 1, 4)
        ).astype(e4m3)
        in_maps.append({"xt": xh, "wd": wd8, "smat": S, "w2": w2h})
    if "nc" not in _CACHE:
        _CACHE["nc"] = _build()
    res = run_bass_kernel_spmd(_CACHE["nc"], in_maps, core_ids=list(range(N_CORES)))
    last_bass_results = res
    return np.concatenate([res.results[c]["out"] for c in range(N_CORES)], axis=0)
